# revision 16
# baseline (speedup 1.0000x reference)
"""Trainium2 Bass kernel for nn_Criterion_64510408786520.

Math: for x[M,N] f32, y[M] int:
  sq[m]   = sum_j x[m,j]^2
  dist    = sq - 2x + 1, with dist[m, y[m]] sign-flipped
  out     = mean_m logsumexp_j(-dist[m,j])

The flipped element v[m] = sq[m] - 2*x[m,y[m]] + 1 is the strict row max:
for any j != y[m],  (-dist[m,j]) - v[m] = -2*(sq - x[m,j] - x[m,y[m]] + 1)
                  <= -2*((x_j-.5)^2 + (x_y-.5)^2 + .5 + rest) < -1
and with sq ~ N (sum of N squares) the gap is ~2N, so every other
exp(z - max) underflows to exactly 0.0 in fp32 — identical to what the
fp32 reference computes.  Hence out == mean_m (sq[m] - 2*x[m,y[m]] + 1)
bit-for-bit at fp32 resolution.

Kernel strategy (8 cores, data-parallel over rows):
  per core: x_shard [1024, 8192] streamed as large [128, cols] chunks
  (fsplit controls cols; the final row-tile is split into small pieces
  so the last compute barely extends past the last DMA byte).  The
  square + row-sum alternates per chunk between the ACT engine
  (activation(Square, accum_out=)) and the DVE
  (scalar_tensor_tensor((x*1.0)*x, accum_out=)), each with its OWN
  stride-0 broadcast dummy out and accumulator tile — a shared dummy
  WAW-chains every op across engines and serializes them.  Combined
  consumption (~1.1 us/MB) far exceeds the ~410 GB/s 16-engine DMA
  roofline (~2.4 us/MB), so the stream is never compute-gated.
  x[m, y[m]] is gathered on-device by 8 indirect DMAs (element offsets
  precomputed on host from y), and out_g is written from the gpsimd
  ring mid-stream.  Each core returns per-chunk row-sums (split per
  engine: out_sq_a/out_sq_b) + [128, 8] gathered values; host does the
  final O(8k) scalar reduction (the all-reduce mean).  lean_tail
  replaces Tile's teardown: drain, one barrier, semaphore-range clears
  split 5-way across engines (the RANGE_CLEAR retires ~1 sem/115 ns,
  so one engine clearing everything serializes the tail), sem-only
  final barrier.  Measured ~95 us/kernel on idle HW (HBM roofline:
  32 MB/core at ~410 GB/s = 80.5 us stream + ~9 us NEFF entry + ~2 us
  compute/output tail + ~3 us teardown); contended runs show ~+15 us.

The container's walrus build rejects instructions carrying more than
one sync-wait command, which Tile emits freely — _split_multi_waits()
post-processes the BIR to hoist extras onto standalone EventSemaphore
instructions (see below).
"""

import sys

for _p in ("/opt/trn_rl_repo",):
    if _p not in sys.path:
        sys.path.insert(0, _p)

import numpy as np

M, N = 8192, 8192
NCORES = 8
MS = M // NCORES        # 1024 rows per core
P = 128                 # SBUF partitions
T = MS // P             # 8 row-tiles per core

_cache = {}


def _split_multi_waits(nc):
    """The walrus build in this container encodes at most ONE sync-wait
    command per instruction ("Too many sync wait commands" otherwise).
    Tile attaches several waits to one instruction; hoist all but the
    last onto standalone EventSemaphore instructions issued just before,
    on the same engine — semantically identical (in-order dispatch)."""
    from concourse import mybir as mb

    n_split = 0
    for fn in nc.m.functions:
        for blk in fn.blocks:
            out = []
            changed = False
            for inst in blk.instructions:
                si = inst.sync_info
                if si is not None and len(si.on_wait) > 1:
                    waits = list(si.on_wait)
                    for j, w in enumerate(waits[:-1]):
                        ev = mb.InstEventSemaphore(
                            name=f"{inst.name}-sw{j}", ins=[], outs=[]
                        )
                        ev.engine = inst.engine
                        ev.sync_info = mb.SyncInfo(on_wait=[w], on_update=[])
                        nc.register_instruction(ev, overwrite=True)
                        out.append(ev)
                        n_split += 1
                    inst.sync_info = mb.SyncInfo(
                        on_wait=[waits[-1]], on_update=list(si.on_update)
                    )
                    changed = True
                out.append(inst)
            if changed:
                blk.instructions = out
    return n_split


def build_nc(n_dve=0, bufs=18, fsplit=4, bcast_out=True, compute=True,
             rings=("sync",), gather="device", tail_chunks=None,
             lean_tail=False, compute_eng="act"):
    """Per-core kernel.  T row-tiles of [128, N]; each tile is squared +
    row-summed in a single pass (ACT fused activation(Square, accum_out),
    or DVE mul+reduce two-pass for the last `n_dve` tiles).  `fsplit`
    splits each tile's free dim into that many chunks (smaller DMAs +
    compute units).  `bcast_out` discards the elementwise square via a
    stride-0 broadcast out instead of an in-place write."""
    import concourse.bass as bass
    import concourse.tile as tile
    from concourse import mybir

    orig_dab = tile.TileContext._drain_and_barrier
    if lean_tail:
        # Stock tail: drain; full barrier; sem clears; full barrier.
        # The second butterfly re-drains already-idle engines; a
        # sem-only barrier suffices (NRT completion waits for per-engine
        # halt anyway; clears are in-order on their engine).
        from concourse.vector_clock import ScopedClock

        def _dab(self, tick_clock, wait_clock):
            drain_inst = self.nc.sync.drain()
            wait_clock.add_sem_waits(
                drain_inst.ins, ScopedClock({None: tick_clock.global_clock})
            )
            self.nc.all_engine_barrier()
            assert self.sems is not None
            popped = self.nc._tile_sem_poison_stack.pop()
            assert popped is self._sem_poison
            # The RANGE_CLEAR retires ~1 sem / 115 ns on its engine's
            # sequencer; one engine clearing all N sems serializes the
            # whole teardown behind an N*115ns crawl.  Split the range
            # across all five engines so the crawl runs 5-way parallel.
            from concourse.bass import compact_to_ranges

            sems = sorted(
                s.num if hasattr(s, "num") else s
                for s in self.sems.allocated().values()
            )
            engs = [self.nc.gpsimd, self.nc.sync, self.nc.scalar,
                    self.nc.vector, self.nc.tensor]
            k = max(1, (len(sems) + len(engs) - 1) // len(engs))
            for gi, lo in enumerate(range(0, len(sems), k)):
                eng = engs[gi % len(engs)]
                for r in compact_to_ranges(sems[lo : lo + k]):
                    eng.drain(semaphore_range=r)
                    eng.sem_clear(r)
            # bookkeeping clear_and_free_semaphores would have done
            self.nc._state.prepend_free_semaphores(sems)
            for poison_set in self.nc._tile_sem_poison_stack:
                poison_set.update(sems)
            self.nc.all_engine_barrier(sem_only=True)

        tile.TileContext._drain_and_barrier = _dab

    nc = bass.Bass()
    NF = N // fsplit
    # Last row-tile is split into `tail_chunks` pieces (default: same as
    # fsplit) and its final piece halved again, so the last exposed
    # activation after the final DMA byte is short while the bulk keeps
    # large, descriptor-efficient transfers.
    if tail_chunks is None:
        tail_chunks = fsplit
    NT = N // tail_chunks
    n_chunks = (T - 1) * fsplit + tail_chunks + (1 if NT % 2 == 0 else 0)
    x = nc.dram_tensor("x", [MS, N], mybir.dt.float32, kind="ExternalInput")
    offs = nc.dram_tensor("offs", [P, T], mybir.dt.int32, kind="ExternalInput")
    out_g = nc.dram_tensor("out_g", [P, T], mybir.dt.float32, kind="ExternalOutput")

    x_flat = x[:].rearrange("a (b c) -> (a b) c", c=1)

    with tile.TileContext(nc) as tc:
        with (
            tc.tile_pool(name="xin", bufs=bufs) as xpool,
            tc.tile_pool(name="small", bufs=1) as small,
        ):
            if gather == "device":
                offs_sb = small.tile([P, T], mybir.dt.int32)
                # offs load on gpsimd (SWDGE) so the sync HWDGE ring
                # leads with the big x loads.
                nc.gpsimd.dma_start(out=offs_sb[:], in_=offs[:])
            g_sb = small.tile([P, T], mybir.dt.float32)

            def emit_gathers():
                if gather != "device":
                    nc.vector.memset(g_sb[:], 0.0)
                    return
                # HW consumes ONE offset per partition per indirect DMA
                # and copies out-free-size contiguous elements; one gather
                # per column gives each (partition, column) its own offset.
                for t in range(T):
                    nc.gpsimd.indirect_dma_start(
                        out=g_sb[:, t : t + 1],
                        out_offset=None,
                        in_=x_flat,
                        in_offset=bass.IndirectOffsetOnAxis(
                            ap=offs_sb[:, t : t + 1], axis=0
                        ),
                    )

            # Chunk list: (row_tile, col_start, col_count).
            chunks = []
            for t in range(T - 1):
                for f in range(fsplit):
                    chunks.append((t, f * NF, NF))
            t = T - 1
            for f in range(tail_chunks):
                c0 = f * NT
                if f == tail_chunks - 1 and NT % 2 == 0:
                    chunks.append((t, c0, NT // 2))
                    chunks.append((t, c0 + NT // 2, NT // 2))
                else:
                    chunks.append((t, c0, NT))

            # Engine assignment per chunk.  Separate accumulator + dummy
            # tiles PER ENGINE: a single shared broadcast-out dummy makes
            # every compute op WAW-depend on the previous one across
            # engines, fully serializing ACT and DVE (observed: zero
            # overlap in the trace).
            def chunk_on_dve(u, t):
                if compute_eng == "dve":
                    return True
                if compute_eng == "alt":
                    return u % 2 == 1
                return t >= T - n_dve

            dve_flags = [chunk_on_dve(u, t) for u, (t, c0, cn) in enumerate(chunks)]
            n_a = sum(1 for f in dve_flags if not f)
            n_b = sum(1 for f in dve_flags if f)
            out_sq_a = nc.dram_tensor("out_sq_a", [P, max(n_a, 1)],
                                      mybir.dt.float32, kind="ExternalOutput")
            out_sq_b = nc.dram_tensor("out_sq_b", [P, max(n_b, 1)],
                                      mybir.dt.float32, kind="ExternalOutput")
            sq_a = small.tile([P, max(n_a, 1)], mybir.dt.float32)
            sq_b = small.tile([P, max(n_b, 1)], mybir.dt.float32)
            dummy_a = small.tile([P, 1], mybir.dt.float32)
            dummy_b = small.tile([P, 1], mybir.dt.float32)
            if n_a == 0 or not compute:
                nc.scalar.memset(sq_a[:], 0.0)
            if n_b == 0 or not compute:
                nc.vector.memset(sq_b[:], 0.0)
            ia = ib = 0
            for u, (t, c0, cn) in enumerate(chunks):
                x_tile = xpool.tile([P, cn], mybir.dt.float32, tag="xin")
                eng = getattr(nc, rings[u % len(rings)])
                eng.dma_start(
                    out=x_tile[:, :cn],
                    in_=x[t * P : (t + 1) * P, c0 : c0 + cn],
                )
                if not compute:
                    continue
                use_dve = dve_flags[u]
                if use_dve:
                    acc = sq_b[:, ib : ib + 1]
                    dummy = dummy_b
                    ib += 1
                else:
                    acc = sq_a[:, ia : ia + 1]
                    dummy = dummy_a
                    ia += 1
                out_ap = dummy.broadcast_to([P, cn]) if bcast_out else x_tile[:, :cn]
                if use_dve and compute_eng in ("dve", "alt"):
                    # Fused square+row-sum in ONE DVE pass: no act-table
                    # load, no READ_ACCUMULATOR step.
                    # out=(x*1.0)*x, accum_out=sum(out).
                    nc.vector.scalar_tensor_tensor(
                        out=out_ap, in0=x_tile[:, :cn], scalar=1.0,
                        in1=x_tile[:, :cn],
                        op0=mybir.AluOpType.mult, op1=mybir.AluOpType.mult,
                        accum_out=acc,
                    )
                elif use_dve:
                    nc.vector.tensor_mul(
                        out=x_tile[:, :cn], in0=x_tile[:, :cn], in1=x_tile[:, :cn]
                    )
                    nc.vector.tensor_reduce(
                        out=acc, in_=x_tile[:, :cn],
                        axis=mybir.AxisListType.X, op=mybir.AluOpType.add,
                    )
                else:
                    nc.scalar.activation(
                        out=out_ap, in_=x_tile[:, :cn],
                        func=mybir.ActivationFunctionType.Square,
                        accum_out=acc,
                    )
            emit_gathers()
            # Issue each output DMA from the engine that produced the
            # data: same-engine in-order ⇒ no cross-engine sem wait, and
            # out_g fires right after the gathers (~38us), fully hidden
            # under the x stream instead of queued in the tail.
            nc.gpsimd.dma_start(out=out_g[:], in_=g_sb[:])
            nc.scalar.dma_start(out=out_sq_a[:], in_=sq_a[:])
            nc.sync.dma_start(out=out_sq_b[:], in_=sq_b[:])
    tile.TileContext._drain_and_barrier = orig_dab
    _split_multi_waits(nc)
    return nc


def shard_inputs(x, y):
    """Build the 8 per-core input maps from the full x [M,N], y [M]."""
    x = np.ascontiguousarray(np.asarray(x, dtype=np.float32))
    y = np.asarray(y).astype(np.int64)
    in_maps = []
    for c in range(NCORES):
        xs = x[c * MS : (c + 1) * MS]
        ys = y[c * MS : (c + 1) * MS]
        lin = np.arange(MS, dtype=np.int64) * N + ys     # element offsets in shard
        offs = lin.astype(np.int32).reshape(T, P).T      # [P, T]: g[p,t]=row t*P+p
        in_maps.append({"x": xs, "offs": np.ascontiguousarray(offs)})
    return in_maps


def combine(results, host_g_total=None):
    """Host-side all-reduce mean over the 8 cores' partial outputs."""
    total = 0.0
    for c in range(NCORES):
        sq = results[c]["out_sq_a"].astype(np.float64)
        total += sq.sum() + results[c]["out_sq_b"].astype(np.float64).sum()
        total += MS                                      # +1 per row
        if host_g_total is None:
            total += -2.0 * results[c]["out_g"].astype(np.float64).sum()
    if host_g_total is not None:
        total += -2.0 * host_g_total
    return np.float32(total / M)


# Tuned config: square+row-sum alternates between the ACT engine
# (fused activation(Square, accum_out)) and the DVE (fused
# scalar_tensor_tensor (x*1.0)*x with accum_out) per chunk, with
# per-engine accumulator/dummy tiles so the two engines overlap
# (a shared broadcast-out dummy WAW-serializes them); 2 MB chunks
# halve the per-chunk sync/sem overhead vs 1 MB; lean_tail replaces
# Tile's teardown with a 5-way-parallel sem-range clear.
BEST_KWARGS = {"compute_eng": "alt", "lean_tail": True,
               "fsplit": 2, "bufs": 11, "tail_chunks": 4}


def run(x, y, trace=False, build_kwargs=None, **spmd_kwargs):
    from concourse.bass_utils import run_bass_kernel_spmd

    if build_kwargs is None:
        build_kwargs = dict(BEST_KWARGS)
    key = tuple(sorted((build_kwargs or {}).items()))
    if key not in _cache:
        _cache[key] = build_nc(**(build_kwargs or {}))
    nc = _cache[key]
    in_maps = shard_inputs(x, y)
    res = run_bass_kernel_spmd(
        nc, in_maps, list(range(NCORES)), trace=trace, **spmd_kwargs
    )
    host_g_total = None
    if (build_kwargs or {}).get("gather", "device") != "device":
        xf = np.asarray(x, dtype=np.float32)
        yi = np.asarray(y).astype(np.int64)
        host_g_total = xf[np.arange(M), yi].astype(np.float64).sum()
    return combine(res.results, host_g_total), res


def kernel(x, y):
    # The axon-tunneled device occasionally throws a transient
    # NRT_EXEC_UNIT_UNRECOVERABLE / UNAVAILABLE on a run and recovers
    # within ~20 s (observed twice this session) — retry once rather
    # than failing the call.
    import time

    try:
        out, _ = run(x, y, trace=False)
    except Exception:
        time.sleep(20)
        out, _ = run(x, y, trace=False)
    return np.asarray(out, dtype=np.float32)



# revision 19
# speedup vs baseline: 1.0486x; 1.0486x over previous
"""Trainium2 Bass kernel for nn_Criterion_64510408786520.

Math: for x[M,N] f32, y[M] int:
  sq[m]   = sum_j x[m,j]^2
  dist    = sq - 2x + 1, with dist[m, y[m]] sign-flipped
  out     = mean_m logsumexp_j(-dist[m,j])

The flipped element v[m] = sq[m] - 2*x[m,y[m]] + 1 is the strict row max:
for any j != y[m],  (-dist[m,j]) - v[m] = -2*(sq - x[m,j] - x[m,y[m]] + 1)
                  <= -2*((x_j-.5)^2 + (x_y-.5)^2 + .5 + rest) < -1
and with sq ~ N (sum of N squares) the gap is ~2N, so every other
exp(z - max) underflows to exactly 0.0 in fp32 — identical to what the
fp32 reference computes.  Hence out == mean_m (sq[m] - 2*x[m,y[m]] + 1)
bit-for-bit at fp32 resolution.

Kernel strategy (8 cores, data-parallel over rows):
  per core: x_shard [1024, 8192] streamed as large [128, cols] chunks
  (fsplit controls cols; the final row-tile is split into small pieces
  so the last compute barely extends past the last DMA byte).  The
  square + row-sum alternates per chunk between the ACT engine
  (activation(Square, accum_out=)) and the DVE
  (scalar_tensor_tensor((x*1.0)*x, accum_out=)), each with its OWN
  stride-0 broadcast dummy out and accumulator tile — a shared dummy
  WAW-chains every op across engines and serializes them.  Combined
  consumption (~1.1 us/MB) far exceeds the ~410 GB/s 16-engine DMA
  roofline (~2.4 us/MB), so the stream is never compute-gated.
  x[m, y[m]] is gathered on-device by 8 indirect DMAs (element offsets
  precomputed on host from y), and out_g is written from the gpsimd
  ring mid-stream.  Each core returns per-chunk row-sums (split per
  engine: out_sq_a/out_sq_b) + [128, 8] gathered values; host does the
  final O(8k) scalar reduction (the all-reduce mean).  lean_tail
  replaces Tile's teardown: drain, one barrier, semaphore-range clears
  split 5-way across engines (the RANGE_CLEAR retires ~1 sem/115 ns,
  so one engine clearing everything serializes the tail), sem-only
  final barrier.  Measured ~95 us/kernel on idle HW (HBM roofline:
  32 MB/core at ~410 GB/s = 80.5 us stream + ~9 us NEFF entry + ~2 us
  compute/output tail + ~3 us teardown); contended runs show ~+15 us.

The container's walrus build rejects instructions carrying more than
one sync-wait command, which Tile emits freely — _split_multi_waits()
post-processes the BIR to hoist extras onto standalone EventSemaphore
instructions (see below).
"""

import sys

for _p in ("/opt/trn_rl_repo",):
    if _p not in sys.path:
        sys.path.insert(0, _p)

import numpy as np

M, N = 8192, 8192
NCORES = 8
MS = M // NCORES        # 1024 rows per core
P = 128                 # SBUF partitions
T = MS // P             # 8 row-tiles per core

_cache = {}


def _split_multi_waits(nc):
    """The walrus build in this container encodes at most ONE sync-wait
    command per instruction ("Too many sync wait commands" otherwise).
    Tile attaches several waits to one instruction; hoist all but the
    last onto standalone EventSemaphore instructions issued just before,
    on the same engine — semantically identical (in-order dispatch)."""
    from concourse import mybir as mb

    n_split = 0
    for fn in nc.m.functions:
        for blk in fn.blocks:
            out = []
            changed = False
            for inst in blk.instructions:
                si = inst.sync_info
                if si is not None and len(si.on_wait) > 1:
                    waits = list(si.on_wait)
                    for j, w in enumerate(waits[:-1]):
                        ev = mb.InstEventSemaphore(
                            name=f"{inst.name}-sw{j}", ins=[], outs=[]
                        )
                        ev.engine = inst.engine
                        ev.sync_info = mb.SyncInfo(on_wait=[w], on_update=[])
                        nc.register_instruction(ev, overwrite=True)
                        out.append(ev)
                        n_split += 1
                    inst.sync_info = mb.SyncInfo(
                        on_wait=[waits[-1]], on_update=list(si.on_update)
                    )
                    changed = True
                out.append(inst)
            if changed:
                blk.instructions = out
    return n_split


def build_nc(n_dve=0, bufs=18, fsplit=4, bcast_out=True, compute=True,
             rings=("sync",), gather="device", tail_chunks=None,
             lean_tail=False, compute_eng="act"):
    """Per-core kernel.  T row-tiles of [128, N]; each tile is squared +
    row-summed in a single pass (ACT fused activation(Square, accum_out),
    or DVE mul+reduce two-pass for the last `n_dve` tiles).  `fsplit`
    splits each tile's free dim into that many chunks (smaller DMAs +
    compute units).  `bcast_out` discards the elementwise square via a
    stride-0 broadcast out instead of an in-place write."""
    import concourse.bass as bass
    import concourse.tile as tile
    from concourse import mybir

    orig_dab = tile.TileContext._drain_and_barrier
    if lean_tail:
        # Stock tail: drain; full barrier; sem clears; full barrier.
        # The second butterfly re-drains already-idle engines; a
        # sem-only barrier suffices (NRT completion waits for per-engine
        # halt anyway; clears are in-order on their engine).
        from concourse.vector_clock import ScopedClock

        def _dab(self, tick_clock, wait_clock):
            drain_inst = self.nc.sync.drain()
            wait_clock.add_sem_waits(
                drain_inst.ins, ScopedClock({None: tick_clock.global_clock})
            )
            self.nc.all_engine_barrier()
            assert self.sems is not None
            popped = self.nc._tile_sem_poison_stack.pop()
            assert popped is self._sem_poison
            # The RANGE_CLEAR retires ~1 sem / 115 ns on its engine's
            # sequencer; one engine clearing all N sems serializes the
            # whole teardown behind an N*115ns crawl.  Split the range
            # across all five engines so the crawl runs 5-way parallel.
            from concourse.bass import compact_to_ranges

            sems = sorted(
                s.num if hasattr(s, "num") else s
                for s in self.sems.allocated().values()
            )
            engs = [self.nc.gpsimd, self.nc.sync, self.nc.scalar,
                    self.nc.vector, self.nc.tensor]
            k = max(1, (len(sems) + len(engs) - 1) // len(engs))
            for gi, lo in enumerate(range(0, len(sems), k)):
                eng = engs[gi % len(engs)]
                for r in compact_to_ranges(sems[lo : lo + k]):
                    eng.drain(semaphore_range=r)
                    eng.sem_clear(r)
            # bookkeeping clear_and_free_semaphores would have done
            self.nc._state.prepend_free_semaphores(sems)
            for poison_set in self.nc._tile_sem_poison_stack:
                poison_set.update(sems)
            self.nc.all_engine_barrier(sem_only=True)

        tile.TileContext._drain_and_barrier = _dab

    nc = bass.Bass()
    NF = N // fsplit
    # Last row-tile is split into `tail_chunks` pieces (default: same as
    # fsplit) and its final piece halved again, so the last exposed
    # activation after the final DMA byte is short while the bulk keeps
    # large, descriptor-efficient transfers.
    if tail_chunks is None:
        tail_chunks = fsplit
    NT = N // tail_chunks
    n_chunks = (T - 1) * fsplit + tail_chunks + (1 if NT % 2 == 0 else 0)
    x = nc.dram_tensor("x", [MS, N], mybir.dt.float32, kind="ExternalInput")
    offs = nc.dram_tensor("offs", [P, T], mybir.dt.int32, kind="ExternalInput")
    out_g = nc.dram_tensor("out_g", [P, T], mybir.dt.float32, kind="ExternalOutput")

    x_flat = x[:].rearrange("a (b c) -> (a b) c", c=1)

    with tile.TileContext(nc) as tc:
        with (
            tc.tile_pool(name="xin", bufs=bufs) as xpool,
            tc.tile_pool(name="small", bufs=1) as small,
        ):
            if gather == "device":
                offs_sb = small.tile([P, T], mybir.dt.int32)
                # offs load on gpsimd (SWDGE) so the sync HWDGE ring
                # leads with the big x loads.
                nc.gpsimd.dma_start(out=offs_sb[:], in_=offs[:])
            g_sb = small.tile([P, T], mybir.dt.float32)

            def emit_gathers():
                if gather != "device":
                    nc.vector.memset(g_sb[:], 0.0)
                    return
                # HW consumes ONE offset per partition per indirect DMA
                # and copies out-free-size contiguous elements; one gather
                # per column gives each (partition, column) its own offset.
                for t in range(T):
                    nc.gpsimd.indirect_dma_start(
                        out=g_sb[:, t : t + 1],
                        out_offset=None,
                        in_=x_flat,
                        in_offset=bass.IndirectOffsetOnAxis(
                            ap=offs_sb[:, t : t + 1], axis=0
                        ),
                    )

            # Chunk list: (row_tile, col_start, col_count).
            chunks = []
            for t in range(T - 1):
                for f in range(fsplit):
                    chunks.append((t, f * NF, NF))
            t = T - 1
            for f in range(tail_chunks):
                c0 = f * NT
                if f == tail_chunks - 1 and NT % 2 == 0:
                    chunks.append((t, c0, NT // 2))
                    chunks.append((t, c0 + NT // 2, NT // 2))
                else:
                    chunks.append((t, c0, NT))

            # Engine assignment per chunk.  Separate accumulator + dummy
            # tiles PER ENGINE: a single shared broadcast-out dummy makes
            # every compute op WAW-depend on the previous one across
            # engines, fully serializing ACT and DVE (observed: zero
            # overlap in the trace).
            def chunk_on_dve(u, t):
                if compute_eng == "dve":
                    return True
                if compute_eng == "alt":
                    return u % 2 == 1
                return t >= T - n_dve

            dve_flags = [chunk_on_dve(u, t) for u, (t, c0, cn) in enumerate(chunks)]
            n_a = sum(1 for f in dve_flags if not f)
            n_b = sum(1 for f in dve_flags if f)
            out_sq_a = nc.dram_tensor("out_sq_a", [P, max(n_a, 1)],
                                      mybir.dt.float32, kind="ExternalOutput")
            out_sq_b = nc.dram_tensor("out_sq_b", [P, max(n_b, 1)],
                                      mybir.dt.float32, kind="ExternalOutput")
            sq_a = small.tile([P, max(n_a, 1)], mybir.dt.float32)
            sq_b = small.tile([P, max(n_b, 1)], mybir.dt.float32)
            dummy_a = small.tile([P, 1], mybir.dt.float32)
            dummy_b = small.tile([P, 1], mybir.dt.float32)
            if n_a == 0 or not compute:
                nc.scalar.memset(sq_a[:], 0.0)
            if n_b == 0 or not compute:
                nc.vector.memset(sq_b[:], 0.0)
            ia = ib = 0
            for u, (t, c0, cn) in enumerate(chunks):
                x_tile = xpool.tile([P, cn], mybir.dt.float32, tag="xin")
                eng = getattr(nc, rings[u % len(rings)])
                eng.dma_start(
                    out=x_tile[:, :cn],
                    in_=x[t * P : (t + 1) * P, c0 : c0 + cn],
                )
                if not compute:
                    continue
                use_dve = dve_flags[u]
                if use_dve:
                    acc = sq_b[:, ib : ib + 1]
                    dummy = dummy_b
                    ib += 1
                else:
                    acc = sq_a[:, ia : ia + 1]
                    dummy = dummy_a
                    ia += 1
                out_ap = dummy.broadcast_to([P, cn]) if bcast_out else x_tile[:, :cn]
                if use_dve and compute_eng in ("dve", "alt"):
                    # Fused square+row-sum in ONE DVE pass: no act-table
                    # load, no READ_ACCUMULATOR step.
                    # out=(x*1.0)*x, accum_out=sum(out).
                    nc.vector.scalar_tensor_tensor(
                        out=out_ap, in0=x_tile[:, :cn], scalar=1.0,
                        in1=x_tile[:, :cn],
                        op0=mybir.AluOpType.mult, op1=mybir.AluOpType.mult,
                        accum_out=acc,
                    )
                elif use_dve:
                    nc.vector.tensor_mul(
                        out=x_tile[:, :cn], in0=x_tile[:, :cn], in1=x_tile[:, :cn]
                    )
                    nc.vector.tensor_reduce(
                        out=acc, in_=x_tile[:, :cn],
                        axis=mybir.AxisListType.X, op=mybir.AluOpType.add,
                    )
                else:
                    nc.scalar.activation(
                        out=out_ap, in_=x_tile[:, :cn],
                        func=mybir.ActivationFunctionType.Square,
                        accum_out=acc,
                    )
            emit_gathers()
            # Issue each output DMA from the engine that produced the
            # data: same-engine in-order ⇒ no cross-engine sem wait, and
            # out_g fires right after the gathers (~38us), fully hidden
            # under the x stream instead of queued in the tail.
            nc.gpsimd.dma_start(out=out_g[:], in_=g_sb[:])
            nc.scalar.dma_start(out=out_sq_a[:], in_=sq_a[:])
            nc.sync.dma_start(out=out_sq_b[:], in_=sq_b[:])
    tile.TileContext._drain_and_barrier = orig_dab
    _split_multi_waits(nc)
    return nc


def build_nc_raw(fsplit=2, bufs=11, tail_chunks=4):
    """Raw-Bass variant (no TileContext): explicit per-chunk DGE
    completion semaphores, alternate ACT/DVE compute, single final
    barrier.  Skips Tile's extra entry barrier rounds (~2-3 us) and
    its teardown (walrus's injected NEFF exit routine clears the sem
    file anyway)."""
    import concourse.bass as bass
    from concourse import mybir

    nc = bass.Bass()
    NF = N // fsplit
    NT = N // tail_chunks
    x = nc.dram_tensor("x", [MS, N], mybir.dt.float32, kind="ExternalInput")
    offs = nc.dram_tensor("offs", [P, T], mybir.dt.int32, kind="ExternalInput")
    out_g = nc.dram_tensor("out_g", [P, T], mybir.dt.float32, kind="ExternalOutput")

    chunks = []
    for t in range(T - 1):
        for f in range(fsplit):
            chunks.append((t, f * NF, NF))
    t = T - 1
    for f in range(tail_chunks):
        c0 = f * NT
        if f == tail_chunks - 1 and NT % 2 == 0:
            chunks.append((t, c0, NT // 2))
            chunks.append((t, c0 + NT // 2, NT // 2))
        else:
            chunks.append((t, c0, NT))

    dve_flags = [u % 2 == 1 for u in range(len(chunks))]
    n_a = sum(1 for f in dve_flags if not f)
    n_b = sum(1 for f in dve_flags if f)
    out_sq_a = nc.dram_tensor("out_sq_a", [P, n_a], mybir.dt.float32,
                              kind="ExternalOutput")
    out_sq_b = nc.dram_tensor("out_sq_b", [P, n_b], mybir.dt.float32,
                              kind="ExternalOutput")

    x_sb = nc.alloc_sbuf_tensor("x_sb", [P, bufs * NF], mybir.dt.float32).ap()
    sq_a = nc.alloc_sbuf_tensor("sq_a", [P, n_a], mybir.dt.float32).ap()
    sq_b = nc.alloc_sbuf_tensor("sq_b", [P, n_b], mybir.dt.float32).ap()
    dum_a = nc.alloc_sbuf_tensor("dum_a", [P, 1], mybir.dt.float32).ap()
    dum_b = nc.alloc_sbuf_tensor("dum_b", [P, 1], mybir.dt.float32).ap()
    offs_sb = nc.alloc_sbuf_tensor("offs_sb", [P, T], mybir.dt.int32).ap()
    g_sb = nc.alloc_sbuf_tensor("g_sb", [P, T], mybir.dt.float32).ap()
    x_flat = x[:].rearrange("a (b c) -> (a b) c", c=1)

    chunk_sem = [nc.alloc_semaphore(f"ck{u}") for u in range(len(chunks))]
    offs_sem = nc.alloc_semaphore("offs_done")
    act_sem = nc.alloc_semaphore("act_prog")
    dve_sem = nc.alloc_semaphore("dve_prog")
    out_sem = nc.alloc_semaphore("outs_done")

    # gather chain on the gpsimd SWDGE ring.  Queue FIFO issue does NOT
    # imply ordered completion (descriptors spread over 16 HW engines),
    # so out_g must wait on explicit gather-completion sems.
    gat_sem = nc.alloc_semaphore("gat_done")
    nc.gpsimd.dma_start(offs_sb[:, :], offs[:]).then_inc(offs_sem, 16)
    nc.gpsimd.wait_ge(offs_sem, 16)
    for t_ in range(T):
        nc.gpsimd.indirect_dma_start(
            out=g_sb[:, t_ : t_ + 1],
            out_offset=None,
            in_=x_flat,
            in_offset=bass.IndirectOffsetOnAxis(
                ap=offs_sb[:, t_ : t_ + 1], axis=0
            ),
        ).then_inc(gat_sem, 16)
    nc.gpsimd.wait_ge(gat_sem, 16 * T)
    nc.gpsimd.dma_start(out_g[:], g_sb[:, :]).then_inc(out_sem, 16)

    idx_of = []
    ia = ib = 0
    for u, f in enumerate(dve_flags):
        idx_of.append(ib if f else ia)
        if f:
            ib += 1
        else:
            ia += 1

    for u, (t_, c0, cn) in enumerate(chunks):
        slot = u % bufs
        x_ap = x_sb[:, slot * NF : slot * NF + cn]
        if u >= bufs:
            v = u - bufs
            sem = dve_sem if dve_flags[v] else act_sem
            nc.sync.wait_ge(sem, idx_of[v] + 1)
        nc.sync.dma_start(x_ap, x[t_ * P : (t_ + 1) * P, c0 : c0 + cn]).then_inc(
            chunk_sem[u], 16
        )
        if dve_flags[u]:
            nc.vector.wait_ge(chunk_sem[u], 16)
            nc.vector.scalar_tensor_tensor(
                out=dum_b.broadcast_to([P, cn]), in0=x_ap, scalar=1.0,
                in1=x_ap, op0=mybir.AluOpType.mult, op1=mybir.AluOpType.mult,
                accum_out=sq_b[:, idx_of[u] : idx_of[u] + 1],
            ).then_inc(dve_sem, 1)
        else:
            nc.scalar.wait_ge(chunk_sem[u], 16)
            nc.scalar.activation(
                out=dum_a.broadcast_to([P, cn]), in_=x_ap,
                func=mybir.ActivationFunctionType.Square,
                accum_out=sq_a[:, idx_of[u] : idx_of[u] + 1],
            ).then_inc(act_sem, 1)

    # outputs: ACT's from its own queue (in-order), DVE's via sync
    nc.scalar.dma_start(out_sq_a[:], sq_a[:, :]).then_inc(out_sem, 16)
    nc.sync.wait_ge(dve_sem, n_b)
    nc.sync.dma_start(out_sq_b[:], sq_b[:, :]).then_inc(out_sem, 16)
    nc.sync.wait_ge(out_sem, 48)
    nc.all_engine_barrier()
    _split_multi_waits(nc)
    return nc


def shard_inputs(x, y):
    """Build the 8 per-core input maps from the full x [M,N], y [M]."""
    x = np.ascontiguousarray(np.asarray(x, dtype=np.float32))
    y = np.asarray(y).astype(np.int64)
    in_maps = []
    for c in range(NCORES):
        xs = x[c * MS : (c + 1) * MS]
        ys = y[c * MS : (c + 1) * MS]
        lin = np.arange(MS, dtype=np.int64) * N + ys     # element offsets in shard
        offs = lin.astype(np.int32).reshape(T, P).T      # [P, T]: g[p,t]=row t*P+p
        in_maps.append({"x": xs, "offs": np.ascontiguousarray(offs)})
    return in_maps


def combine(results, host_g_total=None):
    """Host-side all-reduce mean over the 8 cores' partial outputs."""
    total = 0.0
    for c in range(NCORES):
        sq = results[c]["out_sq_a"].astype(np.float64)
        total += sq.sum() + results[c]["out_sq_b"].astype(np.float64).sum()
        total += MS                                      # +1 per row
        if host_g_total is None:
            total += -2.0 * results[c]["out_g"].astype(np.float64).sum()
    if host_g_total is not None:
        total += -2.0 * host_g_total
    return np.float32(total / M)


# Tuned config: square+row-sum alternates between the ACT engine
# (fused activation(Square, accum_out)) and the DVE (fused
# scalar_tensor_tensor (x*1.0)*x with accum_out) per chunk, with
# per-engine accumulator/dummy tiles so the two engines overlap
# (a shared broadcast-out dummy WAW-serializes them); 2 MB chunks
# halve the per-chunk sync/sem overhead vs 1 MB; lean_tail replaces
# Tile's teardown with a 5-way-parallel sem-range clear.
BEST_KWARGS = {"compute_eng": "alt", "lean_tail": True,
               "fsplit": 2, "bufs": 11, "tail_chunks": 4}


def run(x, y, trace=False, build_kwargs=None, **spmd_kwargs):
    from concourse.bass_utils import run_bass_kernel_spmd

    if build_kwargs is None:
        build_kwargs = dict(BEST_KWARGS)
    key = tuple(sorted((build_kwargs or {}).items()))
    if key not in _cache:
        bk = dict(build_kwargs or {})
        if bk.pop("raw", False):
            _cache[key] = build_nc_raw(**bk)
        else:
            _cache[key] = build_nc(**bk)
    nc = _cache[key]
    in_maps = shard_inputs(x, y)
    res = run_bass_kernel_spmd(
        nc, in_maps, list(range(NCORES)), trace=trace, **spmd_kwargs
    )
    host_g_total = None
    if (build_kwargs or {}).get("gather", "device") != "device":
        xf = np.asarray(x, dtype=np.float32)
        yi = np.asarray(y).astype(np.int64)
        host_g_total = xf[np.arange(M), yi].astype(np.float64).sum()
    return combine(res.results, host_g_total), res


def kernel(x, y):
    # The axon-tunneled device occasionally throws a transient
    # NRT_EXEC_UNIT_UNRECOVERABLE / UNAVAILABLE on a run and recovers
    # within ~20 s (observed twice this session) — retry once rather
    # than failing the call.
    import time

    try:
        out, _ = run(x, y, trace=False)
    except Exception:
        time.sleep(20)
        out, _ = run(x, y, trace=False)
    return np.asarray(out, dtype=np.float32)

